# revision 1
# baseline (speedup 1.0000x reference)
"""Viterbi CRF decode (B=512, T=1024, L=48) on 8 Trainium2 NeuronCores.

Data-parallel over batch: 64 batches per core. On-core layout packs the
64 batches onto 128 SBUF partitions as (batch, half) pairs p = 2b + h;
partition (b, h) computes the Viterbi recurrence for output tags
j in [24h, 24h+24) and holds the full 48-entry v vector in
"own-half-first" rotated order, so every instruction uses
partition-uniform access patterns (all h-dependence lives in
precomputed constant tables).

Forward: the serial chain (scores = v_bcast + transT, blocked
reduce-max -> new v, + emission, pair-swap shuffle) runs on VectorE.
Backpointer extraction (is_equal + iota-mult + reduce-max,
first-occurrence/tie-exact, stored in "R-space" = 48 - tag) is batched
over KB steps via score/max history tiles and partially offloaded to
GPSIMD, off the critical path. Backtrack: one-hot select chain, 3 small
VectorE ops per step over chunk-reassembled bp history.
"""

import sys

for _p in ("/opt/trn_rl_repo",):
    if _p not in sys.path:
        sys.path.insert(0, _p)

import numpy as np

import concourse.bacc as bacc
import concourse.tile as tile
from concourse import mybir
from concourse.bass_utils import run_bass_kernel_spmd

B, T, L = 512, 1024, 48
LH = L // 2  # 24: tags per partition
NCORES = 8
BL = B // NCORES  # 64 batches per core
P = 2 * BL  # 128 partitions
F32 = mybir.dt.float32
BF16 = mybir.dt.bfloat16


BIGNEG = 1.0e9


def build_program(bl=BL, t_len=T, debug=False, kb=4, j1=24, c_mode="dve",
                  we=64, wb=64, skip_bp=False, skip_bt=False, bt_mode="3op",
                  use_bf16=True, eq_bufs=2, a_split=0, sch_bufs=2):
    """Per-core Bass program. kb: bp-extraction batch depth. c_mode:
    'gp_stt' = bp masking via GPSIMD sub + scalar_tensor_tensor (DELTA
    trick); 'dve' = classic is_equal on DVE + mult split at j1 (j' < j1
    on GPSIMD, rest on VectorE)."""
    p = 2 * bl
    nc = bacc.Bacc("TRN2", target_bir_lowering=False, debug=debug)

    emis = nc.dram_tensor("emis", [p, t_len, LH], F32, kind="ExternalInput")
    v0 = nc.dram_tensor("v0", [p, L], F32, kind="ExternalInput")
    transt4 = nc.dram_tensor("transt4", [p, LH, L], F32, kind="ExternalInput")
    iotarev = nc.dram_tensor("iotarev", [p, L], F32, kind="ExternalInput")
    jm2 = nc.dram_tensor("jm2", [p, L], F32, kind="ExternalInput")
    endrep = nc.dram_tensor("endrep", [p, L], F32, kind="ExternalInput")
    paths_out = nc.dram_tensor("paths", [p, t_len], mybir.dt.int32,
                               kind="ExternalOutput")

    BP16 = BF16 if use_bf16 else F32
    we = min(we, t_len)
    wb = min(wb, t_len)  # backtrack chunk width
    swap = [(i ^ 1) for i in range(32)]

    with tile.TileContext(nc) as tc:
        with (
            tc.tile_pool(name="consts", bufs=1) as consts,
            tc.tile_pool(name="hist", bufs=1) as hist,
            tc.tile_pool(name="echunks", bufs=2) as echunks,
            tc.tile_pool(name="sch", bufs=sch_bufs) as schpool,
            tc.tile_pool(name="eqp", bufs=eq_bufs) as eqpool,
            tc.tile_pool(name="work", bufs=1) as work,
            tc.tile_pool(name="vf", bufs=2) as vfpool,
            tc.tile_pool(name="bt", bufs=1) as btpool,
        ):
            tt4 = consts.tile([p, LH, L], F32)
            nc.sync.dma_start(out=tt4, in_=transt4.ap())
            ior = consts.tile([p, L], F32)
            nc.sync.dma_start(out=ior, in_=iotarev.ap())
            jm2t = consts.tile([p, L], F32)
            nc.sync.dma_start(out=jm2t, in_=jm2.ap())
            endt = consts.tile([p, L], F32)
            nc.sync.dma_start(out=endt, in_=endrep.ap())

            bph = hist.tile([p, t_len - 1, LH], BP16)  # bp history, R-space
            paths = hist.tile([p, t_len], F32)  # R-space tags

            vcur = vfpool.tile([p, L], F32, tag="vf")
            nc.sync.dma_start(out=vcur, in_=v0.ap())

            # ---------------- forward ----------------
            def flush_bp(sch, pmh, kn, t0):
                """Extract bp for steps t0..t0+kn-1 (bph rows t0-1..)."""
                eq = eqpool.tile([p, kb, LH, L], F32, tag="eq")
                eq3 = eq[:, :kn].rearrange("p k j i -> p (k j) i")
                sch3 = sch[:, :kn].rearrange("p k j i -> p (k j) i")
                pm_b3 = (pmh[:, :kn, :].rearrange("p k j -> p (k j)")
                         .unsqueeze(2).broadcast_to([p, kn * LH, L]))
                ior_b3 = (ior[:, :].unsqueeze(1)
                          .broadcast_to([p, kn * LH, L]))
                pm_b = (pmh[:, :kn, :].unsqueeze(3)
                        .broadcast_to([p, kn, LH, L]))
                if c_mode == "gp_stt":
                    # MQ = IOR - BIG*(PM - SC): exact-zero delta keeps IOR,
                    # any nonzero delta (>= ulp) drops below every IOR.
                    nc.gpsimd.tensor_sub(out=eq3, in0=pm_b3, in1=sch3)
                    nc.vector.scalar_tensor_tensor(
                        out=eq3, in0=eq3, scalar=-BIGNEG,
                        in1=ior_b3, op0=mybir.AluOpType.mult,
                        op1=mybir.AluOpType.add)
                else:
                    nc.vector.tensor_tensor(out=eq[:, :kn], in0=sch[:, :kn],
                                            in1=pm_b,
                                            op=mybir.AluOpType.is_equal)
                    ior_b1 = (ior[:, :].unsqueeze(1).unsqueeze(1)
                              .broadcast_to([p, kn, j1, L]))
                    nc.gpsimd.tensor_mul(out=eq[:, :kn, 0:j1, :],
                                         in0=eq[:, :kn, 0:j1, :], in1=ior_b1)
                    if j1 < LH:
                        ior_b2 = (ior[:, :].unsqueeze(1).unsqueeze(1)
                                  .broadcast_to([p, kn, LH - j1, L]))
                        nc.vector.tensor_mul(out=eq[:, :kn, j1:LH, :],
                                             in0=eq[:, :kn, j1:LH, :],
                                             in1=ior_b2)
                nc.vector.tensor_reduce(out=bph[:, t0 - 1 : t0 - 1 + kn, :],
                                        in_=eq[:, :kn],
                                        axis=mybir.AxisListType.X,
                                        op=mybir.AluOpType.max)

            e_tile = None
            sch = pmh = None
            t0 = 1
            for t in range(1, t_len):
                if (t - 1) % we == 0:
                    t1 = min(t + we, t_len)
                    e_tile = echunks.tile([p, we, LH], F32, tag="e")
                    nc.sync.dma_start(out=e_tile[:, : t1 - t, :],
                                      in_=emis.ap()[:, t:t1, :])
                k = (t - 1) % kb
                if k == 0:
                    t0 = t
                    sch = schpool.tile([p, kb, LH, L], F32, tag="sch")
                    pmh = schpool.tile([p, kb, LH], F32, tag="pmh")
                if a_split > 0:
                    v_b1 = (vcur[:, :].unsqueeze(1)
                            .broadcast_to([p, a_split, L]))
                    nc.gpsimd.tensor_add(out=sch[:, k, 0:a_split, :],
                                         in0=v_b1, in1=tt4[:, 0:a_split, :])
                    v_b2 = (vcur[:, :].unsqueeze(1)
                            .broadcast_to([p, LH - a_split, L]))
                    nc.vector.tensor_add(out=sch[:, k, a_split:LH, :],
                                         in0=v_b2, in1=tt4[:, a_split:LH, :])
                else:
                    v_b = vcur[:, :].unsqueeze(1).broadcast_to([p, LH, L])
                    nc.vector.tensor_add(out=sch[:, k], in0=v_b, in1=tt4)
                nc.vector.tensor_reduce(out=pmh[:, k, :], in_=sch[:, k],
                                        axis=mybir.AxisListType.X,
                                        op=mybir.AluOpType.max)
                vnext = vfpool.tile([p, L], F32, tag="vf")
                nc.vector.tensor_add(out=vnext[:, 0:LH], in0=pmh[:, k, :],
                                     in1=e_tile[:, (t - 1) % we, :])
                nc.vector.stream_shuffle(out=vnext[:, LH:L],
                                         in_=vnext[:, 0:LH], mask=swap)
                vcur = vnext
                if (k == kb - 1 or t == t_len - 1) and not skip_bp:
                    flush_bp(sch, pmh, k + 1, t0)

            # ---------------- final tag (tie-exact) ----------------
            vfin = work.tile([p, L], F32)
            nc.vector.tensor_add(out=vfin, in0=vcur, in1=endt)
            mfin = work.tile([p, 1], F32)
            nc.vector.tensor_reduce(out=mfin, in_=vfin,
                                    axis=mybir.AxisListType.X,
                                    op=mybir.AluOpType.max)
            eqf = work.tile([p, L], F32)
            nc.vector.tensor_tensor(out=eqf, in0=vfin,
                                    in1=mfin.broadcast_to([p, L]),
                                    op=mybir.AluOpType.is_equal)
            nc.vector.tensor_mul(out=eqf, in0=eqf, in1=ior)
            nc.vector.tensor_reduce(out=paths[:, t_len - 1 : t_len],
                                    in_=eqf, axis=mybir.AxisListType.X,
                                    op=mybir.AluOpType.max)

            # ---------------- backtrack ----------------
            nchunks = 0 if (skip_bt or skip_bp) else (t_len - 1 + wb - 1) // wb
            jm2v = jm2t[:, :].rearrange("p (s k) -> p s k", s=2)
            for c in range(nchunks - 1, -1, -1):
                c0 = c * wb
                c1 = min(c0 + wb, t_len - 1)
                wn = c1 - c0
                bpf = btpool.tile([p, 2, wb, LH], BP16, tag="bpf")
                nc.vector.tensor_copy(out=bpf[:, 0, :wn, :],
                                      in_=bph[:, c0:c1, :])
                nc.vector.stream_shuffle(out=bpf[:, 1, :wn, :],
                                         in_=bph[:, c0:c1, :], mask=swap)
                for t in range(c1 - 1, c0 - 1, -1):
                    # onehot(jm2 == R_{t+1}); selected bp (others exactly 0).
                    oh = work.tile([p, 2, LH], BP16, tag="oh")
                    nc.vector.tensor_scalar(
                        out=oh, in0=jm2v, scalar1=paths[:, t + 1 : t + 2],
                        scalar2=None, op0=mybir.AluOpType.is_equal)
                    if bt_mode == "ttr":
                        mq = work.tile([p, 2, LH], BP16, tag="mq")
                        nc.vector.tensor_tensor_reduce(
                            out=mq, in0=oh, in1=bpf[:, :, t - c0, :],
                            scale=1.0, scalar=0.0,
                            op0=mybir.AluOpType.mult,
                            op1=mybir.AluOpType.max,
                            accum_out=paths[:, t : t + 1], opt_aps=False)
                    else:
                        mq = work.tile([p, 2, LH], BP16, tag="mq")
                        nc.vector.tensor_mul(out=mq, in0=oh,
                                             in1=bpf[:, :, t - c0, :])
                        nc.vector.tensor_reduce(
                            out=paths[:, t : t + 1], in_=mq,
                            axis=mybir.AxisListType.XY,
                            op=mybir.AluOpType.max)

            # ---------------- output: tag = 48 - R, cast int32 ----------
            tagi = hist.tile([p, t_len], mybir.dt.int32)
            nc.vector.tensor_scalar(out=tagi, in0=paths, scalar1=-1.0,
                                    scalar2=float(L),
                                    op0=mybir.AluOpType.mult,
                                    op1=mybir.AluOpType.add)
            nc.sync.dma_start(out=paths_out.ap(), in_=tagi)

    nc.compile()
    return nc


def make_core_inputs(emissions, transitions, start_transitions,
                     end_transitions, bl=BL, t_len=T, ncores=NCORES):
    """Host-side prep: per-core input dicts (numpy, all fp32)."""
    p = 2 * bl
    harr = np.arange(p) % 2
    barr = np.arange(p) // 2
    gi = (np.arange(L)[None, :] + LH * harr[:, None]) % L  # [p, L]
    gj = LH * harr[:, None] + np.arange(LH)[None, :]  # [p, LH]
    tt4 = transitions[gi[:, None, :], gj[:, :, None]].astype(np.float32)
    iotarev = (L - gi).astype(np.float32)
    k = np.arange(L)[None, :]
    j_of = np.where(k < LH, LH * harr[:, None] + k,
                    LH * (1 - harr[:, None]) + (k - LH))
    jm2 = (L - j_of).astype(np.float32)
    endrep = end_transitions[gi].astype(np.float32)

    in_maps = []
    for c in range(ncores):
        em = emissions[c * bl : (c + 1) * bl, :t_len]  # [bl, t, L]
        e_pre = np.ascontiguousarray(
            em.reshape(bl, t_len, 2, LH).transpose(0, 2, 1, 3)
            .reshape(p, t_len, LH))
        vfull = (start_transitions[None, :] + em[:, 0]).astype(np.float32)
        v0 = vfull[barr[:, None], gi]
        in_maps.append({
            "emis": e_pre,
            "v0": np.ascontiguousarray(v0),
            "transt4": tt4,
            "iotarev": iotarev,
            "jm2": jm2,
            "endrep": endrep,
        })
    return in_maps


_prog_cache = {}
_run_opts = {"trace": False}
_last_result = None


def kernel(emissions, mask, transitions, start_transitions, end_transitions):
    global _last_result
    emissions = np.asarray(emissions, dtype=np.float32)
    transitions = np.asarray(transitions, dtype=np.float32)
    start_transitions = np.asarray(start_transitions, dtype=np.float32)
    end_transitions = np.asarray(end_transitions, dtype=np.float32)

    key = (BL, T)
    if key not in _prog_cache:
        _prog_cache[key] = build_program()
    nc = _prog_cache[key]

    in_maps = make_core_inputs(emissions, transitions, start_transitions,
                               end_transitions)
    res = run_bass_kernel_spmd(nc, in_maps, core_ids=list(range(NCORES)),
                               trace=_run_opts["trace"])
    _last_result = res
    outs = [r["paths"][::2, :] for r in res.results]  # h=0 partitions
    return np.concatenate(outs, axis=0).astype(np.int32)


if __name__ == "__main__":
    pass

